# revision 61
# baseline (speedup 1.0000x reference)
"""Trainium2 Bass kernel for nn_MemoryAsContextTitan.

Sharding: batch b (B=4) runs on the core pair (2b, 2b+1); the MHA heads are
tensor-parallel across the pair (even core: heads 0-3, odd core: heads 4-7 —
divergence purely via per-core weight inputs, the program is SPMD-identical).
Each chunk AllGathers the two o2 halves (per-hp2 pieces so the first gather
hides under the second head-pair's compute); ao_w rows are per-core reordered
[local heads; peer heads] so the out-projection starts on the local half
during the collective, and the remote half is merged with per-core 0/1 masks.
Everything else (retrieves, EMA memory, out-proj epilogue) is replicated in
the pair. Per core everything is SBUF-resident; activations are feature-major
[D, tokens]. Softmax without max-subtraction (scores provably < 9 here);
MHA denominators via a ones-column in V summed by the av matmul itself;
normalize chains: scalar-engine bf16 copy of the raw denominator row ->
ones-matmul broadcast -> fast custom-DVE reciprocal. The EMA memory update
keeps an unscaled running sum (0.9^c folded into the k/v projection
epilogues).
"""

import functools
import numpy as np

B, S, D = 4, 3968, 512
H, HD = 8, 64
CHUNK, NPM, MEM = 496, 32, 1024
NCH = S // CHUNK  # 8
KT = D // 128     # 4
HT = KT // 2      # 2 local head-pair blocks per core (4 heads)
HL = D // 2       # 256 local head dims
MT = MEM // 128   # 8
ISD = float(1.0 / np.float32(np.sqrt(D)))
ISH = float(1.0 / np.float32(np.sqrt(HD)))

WN = ["mq", "mk", "mv"]
WS = ["qp"]
WH = ["aq", "ak", "av"]  # per-core head-half weights [D, HL]


def _program():
    import concourse.bass as bass
    import concourse.mybir as mybir
    import concourse.tile as tile
    from concourse import bacc
    from contextlib import ExitStack

    f32 = mybir.dt.float32
    f32r = mybir.dt.float32r
    bf16 = mybir.dt.bfloat16
    Alu = mybir.AluOpType
    Act = mybir.ActivationFunctionType

    def r(ap):
        return ap

    nc = bacc.Bacc("TRN2", target_bir_lowering=False, debug=False)

    xT = nc.dram_tensor("xT", [NCH, D, CHUNK], bf16, kind="ExternalInput").ap()
    pmT = nc.dram_tensor("pmT", [D, NPM], bf16, kind="ExternalInput").ap()
    wd = {n: nc.dram_tensor(f"w_{n}", [D, D], bf16, kind="ExternalInput").ap()
          for n in WN + WS}
    # per-core head-half weights: even core heads 0-3, odd core heads 4-7
    for n in WH:
        wd[n] = nc.dram_tensor(f"w_{n}", [D, HL], bf16,
                               kind="ExternalInput").ap()
    waon_d = nc.dram_tensor("w_aon", [D, D], bf16, kind="ExternalInput").ap()
    bd = {n: nc.dram_tensor(f"b_{n}", [128, KT], f32, kind="ExternalInput").ap()
          for n in ["qp", "mk", "mv", "mq", "ao"]}
    for n in ["aq", "ak"]:
        bd[n] = nc.dram_tensor(f"b_{n}", [128, HT], f32,
                               kind="ExternalInput").ap()
    bbd = {n: nc.dram_tensor(f"bb_{n}", [128, D], f32, kind="ExternalInput").ap()
           for n in ["mv", "ao"]}
    bbd["av"] = nc.dram_tensor("bb_av", [128, HL], f32,
                               kind="ExternalInput").ap()
    # per-core merge masks: o2 peer half = g0*msel[0] + g1*msel[1]
    msel_d = nc.dram_tensor("msel", [128, 2], f32, kind="ExternalInput").ap()
    # feature-major output [D, CHUNK] per chunk; host transposes
    out_d = nc.dram_tensor("out", [NCH, D, CHUNK], f32, kind="ExternalOutput").ap()
    RG = [[0, 1], [2, 3], [4, 5], [6, 7]]  # core pair replica groups

    with nc.allow_low_precision(reason="bf16 attention pipeline, fp32 psum"), \
            tile.TileContext(nc) as tc, ExitStack() as ctx:
        wp = ctx.enter_context(tc.tile_pool(name="wp", bufs=1))
        sp = ctx.enter_context(tc.tile_pool(name="sp", bufs=1))
        ap_ = ctx.enter_context(tc.tile_pool(name="act", bufs=2))
        php = ctx.enter_context(tc.tile_pool(name="php", bufs=3))
        smp = ctx.enter_context(tc.tile_pool(name="smp", bufs=2))
        wsp = ctx.enter_context(tc.tile_pool(name="wsp", bufs=2))
        ps = ctx.enter_context(tc.tile_pool(name="ps", bufs=2, space="PSUM"))
        dcc = ctx.enter_context(tc.tile_pool(name="dcc", bufs=4, space="DRAM"))

        def wstream(n, c, width=D, eng=None):
            t = wsp.tile([128, KT * width], bf16, name=f"wst_{n}{c}",
                         tag="wstream")
            for kt in range(KT):
                (eng or nc.sync).dma_start(
                    out=t[:, kt * width:(kt + 1) * width],
                    in_=wd[n][kt * 128:(kt + 1) * 128, :])
            return t

        # biases first: chunk-0's epilogues need them, the big weight tiles
        # are only consumed later
        bia = {}
        for n in bd:
            bia[n] = wp.tile([128, HT if n in ("aq", "ak") else KT], f32,
                             name=f"bs_{n}")
            nc.gpsimd.dma_start(out=bia[n][:], in_=bd[n][:, :])
        bb = {}
        for n in bbd:
            bb[n] = wp.tile([128, HL if n == "av" else D], f32,
                            name=f"bbs_{n}")
            nc.gpsimd.dma_start(out=bb[n][:], in_=bbd[n][:, :])
        w = {}
        for n in WN:
            w[n] = wp.tile([128, KT * D], bf16, name=f"ws_{n}")
            for kt in range(KT):
                nc.gpsimd.dma_start(out=w[n][:, kt * D:(kt + 1) * D],
                                  in_=wd[n][kt * 128:(kt + 1) * 128, :])
        waon = wp.tile([128, KT * D], bf16, name="ws_aon")
        for kt in range(KT):
            nc.gpsimd.dma_start(out=waon[:, kt * D:(kt + 1) * D],
                                in_=waon_d[kt * 128:(kt + 1) * 128, :])
        ones_cb = wp.tile([128, 2], bf16, name="ones_cb")
        nc.vector.memset(ones_cb[:], 1.0)
        ones_r = wp.tile([1, 128], bf16, name="ones_r")
        nc.vector.memset(ones_r[:], 1.0)
        msel = wp.tile([128, 2], f32, name="msel")
        nc.gpsimd.dma_start(out=msel[:], in_=msel_d[:, :])

        def wsl(t, kt, dt, width=D):
            if isinstance(t, str):
                t = w[t]
            return t[:, kt * width + dt * 128: kt * width + dt * 128 + 128]

        memT = sp.tile([128, KT, MEM], f32, name="memT")
        memB = sp.tile([128, KT, MEM], bf16, name="memB")

        def proj_fm(src, c0, c1, wn, bn, nm):
            """dst[128,KT,T] (feature-major) = W^T @ src[:, :, c0:c1] + b."""
            T = c1 - c0
            dst = ap_.tile([128, KT, T], bf16, name=nm, tag="qry", bufs=3)
            for dt in range(KT):
                p = ps.tile([128, 512], f32, name=f"p_{nm}{dt}", tag="proj")
                for kt in range(KT):
                    nc.tensor.matmul(p[:, 0:T], r(wsl(wn, kt, dt)),
                                     r(src[:, kt, c0:c1]),
                                     start=kt == 0, stop=kt == KT - 1)
                nc.vector.tensor_scalar(dst[:, dt, :], p[:, 0:T],
                                        bia[bn][:, dt:dt + 1], None, Alu.add)
            return dst

        front = {}

        def emit_front_dma(c):
            """comb tile + its DMAs — issued early so transfers overlap the
            MHA phase and the qp matmuls can fire the moment the collective
            window opens."""
            comb = ap_.tile([128, KT, MEM], bf16, name=f"comb{c}", tag="big")
            if c == 0:
                # mem == 0  =>  hist rows == mv_b exactly
                for dt in range(KT):
                    nc.vector.tensor_scalar(comb[:, dt, NPM:NPM + CHUNK],
                                            bb["mv"][:, 0:CHUNK], 0.0,
                                            bia["mv"][:, dt:dt + 1],
                                            Alu.mult, Alu.add)
            # sync-queue DMAs: at this emission point (mid-MHA, before any
            # gather fetch is enqueued) sync is clear, and the scalar queue
            # is jammed with casts/epilogues — which previously delayed these
            # loads past the collective window they feed
            wsq = wstream("qp", c) if c > 0 else None
            for kt in range(KT):
                nc.sync.dma_start(out=comb[:, kt, NPM + CHUNK:MEM],
                                  in_=xT[c, kt * 128:(kt + 1) * 128, :])
                nc.sync.dma_start(out=comb[:, kt, 0:NPM],
                                  in_=pmT[kt * 128:(kt + 1) * 128, :])
            front[c] = (comb, wsq)

        def emit_front_qp(c):
            # fused W' = Wq_out @ mq_w (host-precomputed): qp directly from x
            comb, wsq = front[c]
            qp = proj_fm(comb, NPM + CHUNK, MEM, wsq, "qp", f"qpf{c}")
            front[c] = (comb, qp)

        emit_front_dma(0)
        for c in range(NCH):
            smem = 0.9 ** c          # scale of memT entering this chunk
            smem2 = 0.9 ** (c + 1)   # scale after the EMA update
            comb, qp = front.pop(c)

            # ---------------- retrieve 1 -> hist cols of comb ---------------
            if c == 0:
                pass
            else:
                # k/v of retrieve-1 over mem_c are bit-identical to the
                # previous chunk's retrieve-2 projections (same memraw, same
                # folded 0.9^c scale) -- reuse those tiles instead of
                # recomputing 32 matmuls
                kT = prev_k2
                vv = prev_v2
                pavs = [ps.tile([128, 2, 512], f32, name=f"pav{c}{i}",
                                tag="avr", bufs=2) for i in range(2)]
                dn = ps.tile([128, 512], f32, name=f"dn{c}", tag="proj")
                for mt in range(MT):
                    p = ps.tile([128, 512], f32, name=f"psc{c}{mt}", tag="sc")
                    for kt in range(KT):
                        nc.tensor.matmul(p[:, 0:CHUNK],
                                         r(kT[:, kt, mt * 128:mt * 128 + 128]),
                                         r(qp[:, kt, :]),
                                         start=kt == 0, stop=kt == KT - 1)
                    ptm = php.tile([128, MEM], bf16, name=f"pt{c}{mt}",
                                   tag="pth", bufs=6)
                    nc.scalar.activation(ptm[:, 0:CHUNK], p[:, 0:CHUNK],
                                         Act.Exp, scale=ISD)
                    nc.tensor.matmul(dn[0:1, 0:CHUNK], ones_cb[:, 0:1],
                                     ptm[:, 0:CHUNK], start=mt == 0,
                                     stop=mt == MT - 1, skip_group_check=True)
                    for dt in range(KT):
                        nc.tensor.matmul(pavs[dt // 2][:, dt % 2, 0:CHUNK],
                                         vv[:, mt, dt * 128:dt * 128 + 128],
                                         ptm[:, 0:CHUNK], start=mt == 0,
                                         stop=mt == MT - 1,
                                         skip_group_check=True)
                # denom row -> bf16 (scalar engine), broadcast via ones
                # matmul, then one full-width fast reciprocal
                rc = smp.tile([1, 512], bf16, name=f"rc{c}", tag="rc", bufs=1)
                nc.scalar.activation(rc[0:1, 0:CHUNK], dn[0:1, 0:CHUNK],
                                     Act.Copy)
                pb = ps.tile([128, 512], f32, name=f"pb{c}", tag="proj")
                nc.tensor.matmul(pb[:, 0:CHUNK], r(ones_r[0:1, :]),
                                 r(rc[0:1, 0:CHUNK]), start=True, stop=True)
                bcs = smp.tile([128, 512], f32, name=f"bcs{c}", tag="bcs", bufs=1)
                nc.vector.reciprocal_approx_fast(bcs[:, 0:CHUNK], pb[:, 0:CHUNK])
                for dt in range(KT):
                    nc.vector.tensor_tensor(comb[:, dt, NPM:NPM + CHUNK],
                                            pavs[dt // 2][:, dt % 2, 0:CHUNK],
                                            bcs[:, 0:CHUNK], Alu.mult)

            # ------- MHA over combined (local 4 heads of the core pair) ------
            qa = ap_.tile([128, HT, MEM], bf16, name=f"qa{c}", tag="qa", bufs=1)
            ka = ap_.tile([128, HT, MEM], bf16, name=f"ka{c}", tag="kT", bufs=1)
            for dst, wn in ((qa, "aq"), (ka, "ak")):
                wst = wstream(wn, c, HL)
                for dt in range(HT):
                    for hf in range(2):
                        p = ps.tile([128, 512], f32, name=f"p_{wn}{c}{dt}{hf}",
                                    tag="proj")
                        for kt in range(KT):
                            nc.tensor.matmul(
                                p[:], r(wsl(wst, kt, dt, HL)),
                                r(comb[:, kt, hf * 512:hf * 512 + 512]),
                                start=kt == 0, stop=kt == KT - 1)
                        nc.vector.tensor_scalar(
                            dst[:, dt, hf * 512:hf * 512 + 512], p[:],
                            bia[wn][:, dt:dt + 1], None, Alu.add)
            wsv = wstream("av", c, HL)
            va = ap_.tile([128, MT, H // 2, 65], bf16, name=f"va{c}", tag="vv",
                          bufs=1)
            for mt in range(MT):
                p = ps.tile([128, 512], f32, name=f"pva{c}{mt}", tag="proj")
                for kt in range(KT):
                    nc.tensor.matmul(p[:, 0:HL],
                                     r(comb[:, kt, mt * 128:mt * 128 + 128]),
                                     r(wsv[:, kt * HL:(kt + 1) * HL]),
                                     start=kt == 0, stop=kt == KT - 1)
                nc.vector.tensor_tensor(
                    va[:, mt, :, 0:64],
                    p[:, 0:HL].rearrange("p (h e) -> p h e", h=H // 2),
                    bb["av"][:].rearrange("p (h e) -> p h e", h=H // 2),
                    Alu.add)
            nc.gpsimd.memset(va[:, :, :, 64:65], 1.0)
            if c + 1 < NCH:
                emit_front_dma(c + 1)

            o2l = sp.tile([128, HT, MEM], bf16, name=f"o2l{c}", tag="ohl",
                          bufs=1)
            o2r = ap_.tile([128, HT, MEM], bf16, name=f"o2r{c}", tag="ohr",
                           bufs=1)
            gds = []

            def fetch_merge(hp2):
                # pull both gather slots, select the peer's with masks
                g0s = smp.tile([128, MEM], bf16, name=f"g0{c}{hp2}",
                               tag="gst", bufs=4)
                g1s = smp.tile([128, MEM], bf16, name=f"g1{c}{hp2}",
                               tag="gst", bufs=4)
                nc.sync.dma_start(out=g0s[:], in_=gds[hp2][0])
                nc.sync.dma_start(out=g1s[:], in_=gds[hp2][1])
                gm = smp.tile([128, MEM], bf16, name=f"gm{c}{hp2}", tag="gst",
                              bufs=4)
                nc.vector.tensor_scalar(gm[:], g0s[:], msel[:, 0:1], None,
                                        Alu.mult)
                nc.vector.scalar_tensor_tensor(o2r[:, hp2, :], g1s[:],
                                               msel[:, 1:2], gm[:],
                                               Alu.mult, Alu.add)

            for hp2 in range(HT):
                if hp2 == 1:
                    fetch_merge(0)
                # heads 2*hp2 (PE rows 0-63) and 2*hp2+1 (rows 64-127):
                # K=64 matmuls in disjoint row groups
                dth = hp2
                pavr = [ps.tile([128, 2, 512], f32, name=f"pavr{c}{hp2}{i}",
                                tag="avr", bufs=2) for i in range(2)]
                for mt in range(MT):
                    ts = [php.tile([128, MEM], bf16, name=f"pth{c}{hp2}{mt}{e}",
                                   tag="pth", bufs=6) for e in range(2)]
                    for qh in range(2):
                        for e in range(2):
                            hp = e * 64
                            psc = ps.tile([128, 512], f32,
                                          name=f"psa{c}{hp2}{mt}{qh}{e}",
                                          tag="sc")
                            nc.tensor.matmul(
                                psc[:],
                                r(ka[hp:hp + 64, dth, mt * 128:mt * 128 + 128]),
                                r(qa[hp:hp + 64, dth, qh * 512:qh * 512 + 512]),
                                start=True, stop=True)
                            nc.scalar.activation(
                                ts[e][:, qh * 512:qh * 512 + 512],
                                psc[:], Act.Exp, scale=ISH)
                    for e in range(2):
                        h = 2 * hp2 + e
                        for qh in range(2):
                            nc.tensor.matmul(pavr[e][0:65, qh, :],
                                             va[:, mt, h, 0:65],
                                             ts[e][:, qh * 512:qh * 512 + 512],
                                             start=mt == 0, stop=mt == MT - 1,
                                             skip_group_check=True)
                osc = smp.tile([64, MEM], bf16, name=f"osc{c}{hp2}",
                               tag="osc", bufs=2)
                for e in range(2):
                    h = 2 * hp2 + e
                    # hp2=0's normalize is off the collective critical path:
                    # broadcast on gpsimd so the tensor queue runs straight
                    # into hp2=1's scores. hp2=1 keeps the (faster) tensor
                    # ones-matmul broadcast since its o2l half feeds the
                    # AllGather that gates the out-projection.
                    rch = smp.tile([1, MEM], f32 if hp2 == 0 else bf16,
                                   name=f"rch{c}{h}", tag="rch", bufs=2)
                    bch = smp.tile([64, MEM], f32, name=f"bch{c}{h}", tag="bch",
                                   bufs=2)
                    bcb = (smp.tile([64, MEM], f32, name=f"bcb{c}{h}",
                                    tag="bcb", bufs=2) if hp2 == 0 else None)
                    for qh in range(2):
                        nc.scalar.activation(rch[0:1, qh * 512:qh * 512 + 512],
                                             pavr[e][64:65, qh, :], Act.Copy)
                        if hp2 == 0:
                            nc.gpsimd.partition_broadcast(
                                bcb[:, qh * 512:qh * 512 + 512],
                                rch[0:1, qh * 512:qh * 512 + 512])
                            nc.vector.reciprocal_approx_fast(
                                bch[:, qh * 512:qh * 512 + 512],
                                bcb[:, qh * 512:qh * 512 + 512])
                        else:
                            pbc = ps.tile([128, 512], f32,
                                          name=f"pbc{c}{h}{qh}", tag="proj")
                            nc.tensor.matmul(pbc[0:64, :],
                                             r(ones_r[0:1, 0:64]),
                                             r(rch[0:1,
                                                   qh * 512:qh * 512 + 512]),
                                             start=True, stop=True)
                            nc.vector.reciprocal_approx_fast(
                                bch[:, qh * 512:qh * 512 + 512], pbc[0:64, :])
                        dst = (o2l[0:64, hp2, qh * 512:qh * 512 + 512]
                               if e == 0 else osc[:, qh * 512:qh * 512 + 512])
                        nc.vector.tensor_tensor(
                            dst, pavr[e][0:64, qh, :],
                            bch[:, qh * 512:qh * 512 + 512], Alu.mult)
                # partition-shift the odd head into rows 64-127 (scalar
                # queue: sync may be blocked on the slot-0 gather fetch)
                nc.scalar.dma_start(out=o2l[64:128, hp2, :], in_=osc[:, :])
                # ship this head-pair's o2 half to the peer: AllGather within
                # the core pair (slot 0 = even core / heads 0-3)
                pd = dcc.tile([128, MEM], bf16, name=f"o2p{c}{hp2}",
                              tag="ccin", bufs=4)
                gd = dcc.tile([2, 128, MEM], bf16, name=f"o2g{c}{hp2}",
                              tag="ccout", bufs=4)
                nc.gpsimd.dma_start(out=pd[:], in_=o2l[:, hp2, :])
                nc.gpsimd.collective_compute(
                    "AllGather", Alu.bypass, replica_groups=RG,
                    ins=[pd.opt()], outs=[gd.opt()])
                gds.append(gd)

            # out-projection: waon rows are per-core ordered [local heads;
            # peer heads], so kt 0-1 contract with o2l (no collective needed)
            # and kt 2-3 with the mask-merged peer half o2r. Wave A's local
            # matmuls + the next chunk's front qp hide the AllGather latency.
            attT = ap_.tile([128, KT, MEM], bf16, name=f"attT{c}", tag="big")
            # f32 copy of the attended tail (final output product accuracy)
            attTf = smp.tile([128, KT, 512], f32, name=f"attTf{c}", tag="atf",
                             bufs=2)
            # psum plan: dt 0,1 share two avr-tag tiles; dt 2 and 3 borrow
            # the sc/proj banks (idle during the collective window) so all
            # four accumulations stay open without blocking on wave A
            pos = {}

            def outproj_psum(dt, hf):
                if (dt, hf) not in pos:
                    if dt < 2:
                        if dt not in pos:
                            pos[dt] = ps.tile([128, 2, 512], f32,
                                              name=f"po{c}{dt}", tag="avr",
                                              bufs=2)
                        pos[(dt, hf)] = pos[dt][:, hf, :]
                    else:
                        t = ps.tile([128, 512], f32, name=f"po{c}{dt}{hf}",
                                    tag="sc" if dt == 2 else "proj")
                        pos[(dt, hf)] = t[:]
                return pos[(dt, hf)]

            def outproj_part(dts, kts, start, stop):
                for dt in dts:
                    for hf in range(2):
                        p = outproj_psum(dt, hf)
                        for kt in kts:
                            src = (o2l if kt < HT else o2r)
                            nc.tensor.matmul(
                                p, wsl(waon, kt, dt),
                                src[:, kt % HT, hf * 512:hf * 512 + 512],
                                start=start and kt == kts[0],
                                stop=stop and kt == kts[-1],
                                skip_group_check=True)

            def epi_one(dt, hf, on_scalar):
                p = pos[(dt, hf)]
                dst = attT[:, dt, hf * 512:hf * 512 + 512]
                if on_scalar:
                    nc.scalar.activation(dst, p, Act.Identity,
                                         bias=bia["ao"][:, dt:dt + 1])
                else:
                    nc.vector.tensor_scalar(dst, p, bia["ao"][:, dt:dt + 1],
                                            None, Alu.add)

            def epi_tail(dt, on_scalar):
                p = pos[(dt, 1)][:, NPM + CHUNK - 512:MEM - 512]
                if on_scalar:
                    nc.scalar.activation(attTf[:, dt, 0:CHUNK], p,
                                         Act.Identity,
                                         bias=bia["ao"][:, dt:dt + 1])
                else:
                    nc.vector.tensor_scalar(attTf[:, dt, 0:CHUNK], p,
                                            bia["ao"][:, dt:dt + 1],
                                            None, Alu.add)

            fetch_merge(1)
            # waves accumulate every kt except the late slot-1 piece first;
            # wave B's local+slot-0 matmuls add fill for the second gather
            outproj_part((0, 1), (0, 1, 2), True, False)
            if c + 1 < NCH:
                emit_front_qp(c + 1)                     # fills the CC window
            outproj_part((2, 3), (0, 1, 2), True, False)
            outproj_part((0, 1), (3,), False, True)
            outproj_part((2, 3), (3,), False, True)
            # epilogues split across scalar/vector in dependency-priority
            # order: dt3 first (releases the proj-tag psum that qp2 rotates
            # into), then the tail halves qp2 reads, then the rest
            epi_one(3, 1, True)
            epi_one(3, 0, False)
            epi_tail(3, True)
            epi_one(2, 1, False)
            epi_one(1, 1, True)
            epi_one(0, 1, False)
            epi_one(2, 0, True)
            epi_one(1, 0, False)
            epi_one(0, 0, True)
            epi_tail(2, False)
            epi_tail(1, True)
            epi_tail(0, False)

            # ---------------- EMA update (unscaled running sum) --------------
            for dt in range(KT):
                if c == 0:
                    nc.vector.tensor_scalar(memT[:, dt, :], attT[:, dt, :],
                                            0.1 / smem2, None, Alu.mult)
                else:
                    nc.vector.scalar_tensor_tensor(memT[:, dt, :],
                                                   attT[:, dt, :], 0.1 / smem2,
                                                   memT[:, dt, :],
                                                   Alu.mult, Alu.add)

            for dt in range(KT):
                nc.scalar.activation(memB[:, dt, :], memT[:, dt, :], Act.Copy)

            # ---------------- retrieve 2 (tail queries only) -----------------
            qp2 = proj_fm(attT, NPM + CHUNK, MEM, "mq", "mq", f"qp2{c}")
            kT2 = ap_.tile([128, KT, MEM], bf16, name=f"kT2{c}", tag="kT",
                           bufs=1)
            for dt in range(KT):
                for hf in range(2):
                    p = ps.tile([128, 512], f32, name=f"pk2{c}{dt}{hf}",
                                tag="proj")
                    for kt in range(KT):
                        nc.tensor.matmul(
                            p[:], r(wsl("mk", kt, dt)),
                            r(memB[:, kt, hf * 512:hf * 512 + 512]),
                            start=kt == 0, stop=kt == KT - 1)
                    nc.vector.tensor_scalar(kT2[:, dt, hf * 512:hf * 512 + 512],
                                            p[:], smem2,
                                            bia["mk"][:, dt:dt + 1],
                                            Alu.mult, Alu.add)
            v2 = ap_.tile([128, MT, 512], bf16, name=f"v2{c}", tag="vv", bufs=1)
            for mt in range(MT):
                p = ps.tile([128, 512], f32, name=f"pv2{c}{mt}", tag="proj")
                for kt in range(KT):
                    nc.tensor.matmul(p[:],
                                     r(memB[:, kt, mt * 128:mt * 128 + 128]),
                                     r(w["mv"][:, kt * D:(kt + 1) * D]),
                                     start=kt == 0, stop=kt == KT - 1)
                nc.vector.scalar_tensor_tensor(v2[:, mt, :], p[:], smem2,
                                               bb["mv"][:], Alu.mult, Alu.add)
            dn2 = ps.tile([128, 512], f32, name=f"dn2{c}", tag="proj")
            pms = [ps.tile([128, 2, 512], f32, name=f"pmo{c}{i}", tag="avr",
                           bufs=2) for i in range(2)]
            for mt in range(MT):
                p = ps.tile([128, 512], f32, name=f"ps2{c}{mt}", tag="sc")
                for kt in range(KT):
                    nc.tensor.matmul(p[:, 0:CHUNK],
                                     r(kT2[:, kt, mt * 128:mt * 128 + 128]),
                                     r(qp2[:, kt, :]),
                                     start=kt == 0, stop=kt == KT - 1)
                ptm = php.tile([128, MEM], bf16, name=f"pt2{c}{mt}",
                               tag="pth", bufs=6)
                nc.scalar.activation(ptm[:, 0:CHUNK], p[:, 0:CHUNK], Act.Exp,
                                     scale=ISD)
                nc.tensor.matmul(dn2[0:1, 0:CHUNK], ones_cb[:, 0:1],
                                 ptm[:, 0:CHUNK], start=mt == 0,
                                 stop=mt == MT - 1, skip_group_check=True)
                # feature-major AV: mout[d, q] accumulated over memory rows
                for dt in range(KT):
                    nc.tensor.matmul(pms[dt // 2][:, dt % 2, 0:CHUNK],
                                     v2[:, mt, dt * 128:dt * 128 + 128],
                                     ptm[:, 0:CHUNK], start=mt == 0,
                                     stop=mt == MT - 1, skip_group_check=True)
            prev_k2, prev_v2 = kT2, v2
            rc2 = smp.tile([1, 512], bf16, name=f"rc2{c}", tag="rc", bufs=1)
            nc.scalar.activation(rc2[0:1, 0:CHUNK], dn2[0:1, 0:CHUNK],
                                 Act.Copy)
            pb2 = ps.tile([128, 512], f32, name=f"pb2{c}", tag="proj")
            nc.tensor.matmul(pb2[:, 0:CHUNK], r(ones_r[0:1, :]),
                             r(rc2[0:1, 0:CHUNK]), start=True, stop=True)
            bcs2 = smp.tile([128, 512], f32, name=f"bcs2{c}", tag="bcs2",
                            bufs=1)
            nc.vector.reciprocal_approx_fast(bcs2[:, 0:CHUNK],
                                             pb2[:, 0:CHUNK])
            for dt in range(KT):
                ota = smp.tile([128, 512], f32, name=f"ota{c}{dt}", tag="ot",
                               bufs=4)
                nc.vector.tensor_tensor(ota[:, 0:CHUNK],
                                        pms[dt // 2][:, dt % 2, 0:CHUNK],
                                        bcs2[:, 0:CHUNK], Alu.mult)
                ot = smp.tile([128, 512], f32, name=f"ot{c}{dt}", tag="ot2",
                              bufs=4)
                nc.vector.tensor_tensor(ot[:, 0:CHUNK], ota[:, 0:CHUNK],
                                        attTf[:, dt, 0:CHUNK], Alu.mult)
                nc.sync.dma_start(out=out_d[c, dt * 128:dt * 128 + 128, :],
                                  in_=ot[:, 0:CHUNK])

    nc.compile()
    return nc


@functools.lru_cache(maxsize=1)
def _built():
    return _program()


def _prep_core_inputs(inputs, b, half):
    """Inputs for one core: batch b, head-half `half` (0: heads 0-3)."""
    import ml_dtypes
    bf = ml_dtypes.bfloat16
    hs = slice(half * HL, (half + 1) * HL)
    x = np.ascontiguousarray(inputs["x"][b])  # [S, D]
    xT = np.ascontiguousarray(
        x.reshape(NCH, CHUNK, D).transpose(0, 2, 1)).astype(bf)
    im = {"xT": xT,
          "pmT": np.ascontiguousarray(inputs["persistent_memory"].T).astype(bf)}
    for n, src in {"mq": "mq_w", "mk": "mk_w", "mv": "mv_w"}.items():
        im[f"w_{n}"] = np.ascontiguousarray(inputs[src]).astype(bf)
    for n, src in {"aq": "aq_w", "ak": "ak_w", "av": "av_w"}.items():
        im[f"w_{n}"] = np.ascontiguousarray(inputs[src][:, hs]).astype(bf)
    w_qp = (inputs["Wq_out"].astype(np.float64)
            @ inputs["mq_w"].astype(np.float64)).astype(np.float32)
    b_qp = (inputs["bq_out"].astype(np.float64)
            @ inputs["mq_w"].astype(np.float64)
            + inputs["mq_b"].astype(np.float64)).astype(np.float32)
    im["w_qp"] = np.ascontiguousarray(w_qp).astype(bf)
    im["b_qp"] = np.ascontiguousarray(b_qp.reshape(KT, 128).T).astype(np.float32)
    # ao_w rows reordered [local heads; peer heads] for the split out-proj
    oth = slice((1 - half) * HL, (2 - half) * HL)
    im["w_aon"] = np.ascontiguousarray(
        np.concatenate([inputs["ao_w"][hs], inputs["ao_w"][oth]],
                       axis=0)).astype(bf)
    mm = np.zeros((128, 2), np.float32)
    mm[:, 1 - half] = 1.0  # peer's gather slot
    im["msel"] = mm
    for n, src in {"mq": "mq_b", "mk": "mk_b", "mv": "mv_b",
                   "ao": "ao_b"}.items():
        im[f"b_{n}"] = np.ascontiguousarray(
            inputs[src].reshape(KT, 128).T).astype(np.float32)
    for n, src in (("aq", "aq_b"), ("ak", "ak_b")):
        im[f"b_{n}"] = np.ascontiguousarray(
            inputs[src][hs].reshape(HT, 128).T).astype(np.float32)
    for n, src in (("mv", "mv_b"), ("ao", "ao_b")):
        im[f"bb_{n}"] = np.ascontiguousarray(
            np.broadcast_to(inputs[src][None, :], (128, D))).astype(np.float32)
    im["bb_av"] = np.ascontiguousarray(
        np.broadcast_to(inputs["av_b"][hs][None, :],
                        (128, HL))).astype(np.float32)
    return im


def _install_ntff_shim():
    """Register the axon NTFF profile hook if the image's antenv lacks the
    axon_hooks submodule (concourse fetches the hook from there). Backed by
    the same _ntff_profile_via_ctypes the axon boot would have installed."""
    import sys, types
    try:
        from antenv.axon_hooks import get_axon_ntff_profile_hook
        if get_axon_ntff_profile_hook() is not None:
            return True
    except ImportError:
        pass
    try:
        import antenv
        from trn_agent_boot.trn_boot import _ntff_profile_via_ctypes
        hook = _ntff_profile_via_ctypes("/opt/axon/libaxon_pjrt.so")
        m = types.ModuleType("antenv.axon_hooks")
        m._hook = hook
        m.get_axon_ntff_profile_hook = lambda: m._hook

        def _set(h):
            m._hook = h

        m.set_axon_ntff_profile_hook = _set
        sys.modules["antenv.axon_hooks"] = m
        antenv.axon_hooks = m
        return True
    except Exception:
        return False


def _patch_artifact_upload():
    """Artifact upload needs a bucket this environment doesn't have; make it
    non-fatal so the profile post-processing can proceed locally."""
    import concourse.bass_utils as bu
    orig = bu.upload_artifacts
    if getattr(orig, "_safe", False):
        return

    def _safe_upload(tmpdir):
        try:
            return orig(tmpdir)
        except Exception:
            return tmpdir

    _safe_upload._safe = True
    bu.upload_artifacts = _safe_upload


def kernel(**inputs):
    inputs = {k: np.asarray(v) for k, v in inputs.items()}
    nc = _built()
    from concourse.bass_utils import run_bass_kernel_spmd
    # batch b on core pair (2b, 2b+1): even core heads 0-3, odd heads 4-7
    in_maps = [_prep_core_inputs(inputs, c // 2, c % 2) for c in range(8)]
    import kernel as _k
    trace_ok = _install_ntff_shim()
    if trace_ok:
        _patch_artifact_upload()
    res = None
    if trace_ok:
        try:
            res = run_bass_kernel_spmd(nc, in_maps, list(range(8)), trace=True)
        except Exception:
            res = None
    if res is None:
        res = run_bass_kernel_spmd(nc, in_maps, list(range(8)))
    _k.LAST_RESULTS = res
    # device output is [NCH, D, CHUNK] feature-major; batch b on core 2b
    out = np.stack([np.asarray(res.results[2 * b]["out"]).transpose(0, 2, 1)
                    .reshape(S, D) for b in range(B)])
    return out

